# revision 47
# baseline (speedup 1.0000x reference)
"""DeepClusteringLoss Trainium2 kernel.

loss = (||V^T V||_F^2 - 2 ||V^T E||_F^2 + ||E^T E||_F^2) / (B*N)
summed over batch, with E = embeddings.reshape(B, N, D), V =
assignments.reshape(B, N, S), N = F*T.

Sharding: data-parallel over batch; one core per batch element; the host
sums the 8 per-core partials (the scalar "all-reduce") and divides by
B*N.

Per-core pipeline (DMA-bound: 23.07 MB fp32 input; the 16 SDMA engines
sustain ~26 GB/s each => ~55-56 us of engine-busy streaming, measured):
- GLOBAL partition map: partition p owns rows [p*1024, (p+1)*1024).
  Chunk c = column c of every partition = 128 rows.
- ALL streaming is HWDGE (SP + ACT rings) in fp32: HWDGE descriptor
  generation is RTL (no Q7 SWDGE boot delay, and ~15% less SDMA
  engine-busy per byte than SWDGE's descriptor format), every DMA
  sprays all 16 SDMA engines evenly, and since the SBUF AXI ports
  (~27 GB/s/engine) bind on the write side, streaming fp32 instead of
  SWDGE-cast-fp16 costs no engine time.
- E streams as column-slices alternating SP/ACT rings through a
  10-deep fp32 ring buffer: small HEAD slices (16,16,32,32) so the
  first ring-FIFO completions (each completion semaphore fires ~3-4 us
  after its data lands) unblock the first casts/matmuls by ~12 us, a
  64-chunk middle, and a small TAIL taper (48,32,16) to shorten the
  post-last-arrival serial chain.  V rides along as three fp32 pieces
  (96/448/480 chunks) cut at slice boundaries: the tiny first piece at
  the very head of the SP ring.
- Interleave copies cast fp32->fp16 while building chunk-PAIR operands
  [V_2q|E_2q|pad4 | V_2q+1|E_2q+1] (128 x 96 fp16, 48-wide halves so
  pair offsets stay 32B-aligned and matmuls are 96 wide, 11% narrower
  than the 108-wide layout): one 4D-AP DVE cast per slice for E, one
  GpSimd cast for V (GpSimd is otherwise idle, issues no DMAs, and
  keeps the ACT sequencer DMA-only so copy waits never block tail E
  DMA issue).  Even/odd Grams accumulate at PSUM partition bases 0/48.
- All DMAs are issued in a first pass with greedy byte-balancing
  across the two rings (so neither ring finishes late), before any
  compute op is emitted — the in-order ACT sequencer can then never
  block DMA issue behind a compute wait.
- The last four slices (80 chunks) retire as 48-wide SINGLES into a
  second PSUM bank, so the main Gram's stop fires four slices early
  and its odd-block epilogue (partition starts must be 0/32/64: pieces
  [32:64) and [64:92)) fully overlaps the tail.  The even block and
  the singles block both live on partitions 0:44, so they share one
  SBUF tile and ONE final 44-descriptor OUT DMA.  The host reassembles
  the four dumped blocks from their disjoint DRAM bands and reduces to
  the scalar partial in float64 (exact).

Remaining fixed costs (measured): ~4.5us front (preamble + TENSOR_LOAD
+ first descriptor gen), ~55.5us stream (engine 0 carries ~4.5us of
instruction fetch and finishes last; every slice spans all 16 engines,
so its lag gates the tail), ~4us completion-semaphore latency on the
last slice, ~2us final cast/matmul/epilogue chain, ~5.5us Tile
end-of-kernel drain + barrier.
"""

import os
from contextlib import ExitStack

import numpy as np

import concourse.bacc as bacc
import concourse.mybir as mybir
import concourse.tile as tile
from concourse.bass_utils import run_bass_kernel_spmd

B, F, T, D, S = 8, 256, 512, 40, 4
N = F * T              # rows per core (131072)
SD = S + D             # 44 combined features
H = 48                 # half-width: V(4) | E(40) | pad(4); 48*2B = 32B-aligned
PW = 2 * H             # paired-chunk width (96)
P = 128                # partitions
U = N // P             # rows per partition in the global map (1024)
N_CORES = 8

MM_DT_NAME = os.environ.get("KERNEL_MM_DT", "float16")
RING = os.environ.get("KERNEL_RING", "alt")   # "alt" | "sp"
EBUFS = int(os.environ.get("KERNEL_EBUFS", "10"))
WBUFS = int(os.environ.get("KERNEL_WBUFS", "8"))

# E slice plan: small HEAD slices so the first ring-FIFO completions
# (and with them the first casts + matmuls) land by ~7 us instead of
# ~16 us; big uniform middle for line-rate DMA; small TAIL taper so the
# last-slice copy+matmul+epilogue dependency chain is short.
SLICES = [16, 16, 32, 32] + [64] * 11 + [48, 48, 48] + [32, 24, 16, 8]
N_SINGLE = 4           # trailing slices retired via the second PSUM bank
assert sum(SLICES) == U
assert all(ub % 2 == 0 for ub in SLICES)

# V pieces, boundaries aligned to slice edges: a tiny leading piece
# covering the head slices (so the first V-copies are not gated by a
# megabyte-scale V transfer), then two big pieces.
VCUTS = [0, 96, 544, U]

_nc_cache = {}


def _build_nc(key):
    (mm_dt_name, ring_mode, ebufs, wbufs) = key
    mm_dt = getattr(mybir.dt, mm_dt_name)
    f32 = mybir.dt.float32

    nc = bacc.Bacc("TRN2", target_bir_lowering=False, debug=False)
    E = nc.dram_tensor("embeddings", (N, D), f32, kind="ExternalInput")
    V = nc.dram_tensor("assignments", (N, S), f32, kind="ExternalInput")
    OUT = nc.dram_tensor("partial", (PW, 176), f32, kind="ExternalOutput")

    # global-map DRAM views: partition p <- rows [p*U, (p+1)*U)
    e_g = E[:, :].rearrange("(p u) d -> p (u d)", p=P)   # [128, U*D]
    v_g = V[:, :].rearrange("(p u) s -> p (u s)", p=P)   # [128, U*S]

    with tile.TileContext(nc) as tc, ExitStack() as ctx:
        res_pool = ctx.enter_context(tc.tile_pool(name="res", bufs=1))
        e_pool = ctx.enter_context(tc.tile_pool(name="e", bufs=ebufs))
        w_pool = ctx.enter_context(tc.tile_pool(name="w", bufs=wbufs))
        psum_pool = ctx.enter_context(tc.tile_pool(name="ps", bufs=1, space="PSUM"))
        g_ps = psum_pool.tile([PW, PW], f32, tag="g")

        # V as three fp32 HWDGE pieces (separate tiles so early slices
        # depend only on the piece that covers them).  The tiny first
        # piece rides at the very head of the SP ring.
        v_tiles = []
        for j in range(len(VCUTS) - 1):
            lo, hi = VCUTS[j], VCUTS[j + 1]
            v_t = res_pool.tile([P, (hi - lo) * S], f32, tag=f"v{j}")
            v_tiles.append((v_t, lo, hi))

        # The final slice accumulates 48-wide SINGLES into a second
        # PSUM bank, so the main Gram's stop fires one slice early and
        # its (bigger) epilogue overlaps the last slice's casts+matmuls.
        g2_ps = psum_pool.tile([H, H], f32, tag="g2")

        # ---- Phase 1: issue ALL DMAs (greedy byte-balanced across the
        # two HWDGE rings, so neither ring finishes late and delays the
        # tail slices).  With every dma_start issued before any ACT
        # copy is emitted, the ACT sequencer's in-order stream can never
        # block DMA issue behind a compute wait, which frees ACT to
        # share the tail cast work with DVE.
        E_CHUNK_B = D * 4          # E bytes per chunk per partition
        V_CHUNK_B = S * 4
        rb = {id(nc.sync): 0, id(nc.scalar): 0}

        def lighter():
            return nc.sync if rb[id(nc.sync)] <= rb[id(nc.scalar)] else nc.scalar

        nc.sync.dma_start(
            out=v_tiles[0][0][:], in_=v_g[:, VCUTS[0] * S:VCUTS[1] * S])
        rb[id(nc.sync)] += (VCUTS[1] - VCUTS[0]) * V_CHUNK_B

        e_tiles = []
        c0 = 0
        for k, ub in enumerate(SLICES):
            e_t = e_pool.tile([P, ub * D], f32, tag="e")
            eng = lighter() if ring_mode == "alt" else nc.sync
            eng.dma_start(out=e_t[:], in_=e_g[:, c0 * D:(c0 + ub) * D])
            rb[id(eng)] += ub * E_CHUNK_B
            if k == 2:
                eng = lighter()
                eng.dma_start(
                    out=v_tiles[1][0][:],
                    in_=v_g[:, VCUTS[1] * S:VCUTS[2] * S])
                rb[id(eng)] += (VCUTS[2] - VCUTS[1]) * V_CHUNK_B
            elif k == 3:
                eng = lighter()
                eng.dma_start(
                    out=v_tiles[2][0][:],
                    in_=v_g[:, VCUTS[2] * S:VCUTS[3] * S])
                rb[id(eng)] += (VCUTS[3] - VCUTS[2]) * V_CHUNK_B
            e_tiles.append((e_t, c0, ub))
            c0 += ub

        # ---- Phase 2: casts + matmuls.  E casts on DVE, V casts on the
        # otherwise-idle GpSimd.  The last N_SINGLE slices retire as
        # 48-wide singles into the second PSUM bank, so the main Gram's
        # stop fires several slices early and its three-piece epilogue
        # fully overlaps the tail slices' casts and matmuls.
        n_sl = len(SLICES)
        pair = 0
        single_u = 0
        n_single_u = sum(SLICES[n_sl - N_SINGLE:])
        for k, (e_t, c0, ub) in enumerate(e_tiles):
            v_src, vlo, vhi = next(
                vt for vt in v_tiles if vt[1] <= c0 < vt[2])
            assert c0 + ub <= vhi
            vc0 = c0 - vlo
            if k < n_sl - N_SINGLE:
                nq = ub // 2
                w_t = w_pool.tile([P, nq * PW], mm_dt, tag="w")
                # 4D views: one cast per slice fills BOTH halves of
                # every pair.
                w5 = w_t[:].rearrange("p (q h c) -> p q h c", h=2, c=H)
                e3 = e_t[:].rearrange("p (q h d) -> p q h d", h=2, d=D)
                v3 = v_src[:, vc0 * S:(vc0 + ub) * S].rearrange(
                    "p (q h s) -> p q h s", h=2, s=S)
                nc.vector.tensor_copy(w5[:, :, :, S:SD], e3)
                nc.gpsimd.tensor_copy(w5[:, :, :, 0:S], v3)
                for q in range(nq):
                    wq = w_t[:, q * PW:(q + 1) * PW]
                    nc.tensor.matmul(
                        g_ps[:], wq, wq,
                        start=(pair == 0),
                        stop=(k == n_sl - N_SINGLE - 1 and q == nq - 1),
                    )
                    pair += 1
            else:
                w_t = w_pool.tile([P, ub * H], mm_dt, tag="w")
                w5 = w_t[:].rearrange("p (u c) -> p u c", c=H)
                e3 = e_t[:].rearrange("p (u d) -> p u d", d=D)
                v3 = v_src[:, vc0 * S:(vc0 + ub) * S].rearrange(
                    "p (u s) -> p u s", s=S)
                nc.vector.tensor_copy(w5[:, :, S:SD], e3)
                nc.gpsimd.tensor_copy(w5[:, :, 0:S], v3)
                for u in range(ub):
                    wu = w_t[:, u * H:(u + 1) * H]
                    nc.tensor.matmul(
                        g2_ps[:], wu, wu,
                        start=(single_u == 0),
                        stop=(single_u == n_single_u - 1))
                    single_u += 1

        # Epilogue: dump only the two 44x44 diagonal Gram blocks of the
        # PSUM accumulator, each on its own HWDGE ring (SP and ACT) so
        # the descriptor generation for the two OUT transfers runs in
        # parallel; the host adds the blocks and reduces to the scalar
        # partial (exact, in float64) alongside the cross-core sum.
        # Partition-start legality: patterns may start at 0/32/64 and,
        # when starting at 32, cover at most 32 partitions.  The odd
        # Gram block lives at [48:92, 48:92], so dump it as two pieces:
        # rows 48:64 ride a 32-partition access at base 32, rows 64:92
        # a 28-partition access at base 64.
        # The even block and the singles block both live on partitions
        # 0:44, so they ride ONE SBUF tile ([44, 88]) and ONE 44-desc
        # OUT DMA -- removing a ~1us descriptor-gen from the critical
        # end.  Each OUT piece gets a disjoint DRAM column band.
        ep = ctx.enter_context(tc.tile_pool(name="ep", bufs=1))
        eg2_sb = ep.tile([SD, 2 * SD], f32, tag="eg2")
        gl_sb = ep.tile([64, SD], f32, tag="gl")
        gh_sb = ep.tile([92, SD], f32, tag="gh")
        # All epilogue copies on DVE (idle in this window): using
        # nc.scalar.copy here would pull in ACT_TABLE_LOAD, whose ~19KB
        # table fetch rides queue 14 on straggler engine 0.
        nc.vector.tensor_copy(eg2_sb[:, 0:SD], g_ps[0:SD, 0:SD])
        nc.vector.tensor_copy(gl_sb[32:64, :], g_ps[32:64, H:H + SD])
        nc.vector.tensor_copy(gh_sb[64:92, :], g_ps[64:92, H:H + SD])
        nc.scalar.dma_start(out=OUT[32:64, 88:132], in_=gl_sb[32:64, :])
        nc.scalar.dma_start(out=OUT[64:92, 132:176], in_=gh_sb[64:92, :])
        nc.vector.tensor_copy(eg2_sb[:, SD:2 * SD], g2_ps[0:SD, 0:SD])
        nc.sync.dma_start(out=OUT[0:SD, 0:2 * SD], in_=eg2_sb[:])

    nc.finalize()
    return nc


def _get_nc():
    key = (MM_DT_NAME, RING, EBUFS, WBUFS)
    if key not in _nc_cache:
        _nc_cache[key] = _build_nc(key)
    return _nc_cache[key]


def _run(embeddings: np.ndarray, assignments: np.ndarray, trace: bool = False):
    nc = _get_nc()
    in_maps = []
    for i in range(N_CORES):
        in_maps.append({
            "embeddings": np.ascontiguousarray(
                embeddings[i].reshape(N, D).astype(np.float32, copy=False)),
            "assignments": np.ascontiguousarray(
                assignments[i].reshape(N, S).astype(np.float32, copy=False)),
        })
    try:
        res = run_bass_kernel_spmd(
            nc, in_maps, core_ids=list(range(N_CORES)), trace=trace
        )
    except Exception:
        res = run_bass_kernel_spmd(
            nc, in_maps, core_ids=list(range(N_CORES)), trace=trace
        )
    partials = []
    for r in res.results:
        gp = np.asarray(r["partial"], dtype=np.float64)
        # even block + singles block (merged dump), + the odd block's
        # two pieces from their relocated DRAM bands.
        odd = np.empty((SD, SD))
        odd[0:16] = gp[48:64, 88:132]
        odd[16:SD] = gp[64:64 + 28, 132:176]
        G = gp[0:SD, 0:SD] + gp[0:SD, SD:2 * SD] + odd
        bm = G[0:S, S:SD]
        partials.append(np.sum(G * G) - 4.0 * np.sum(bm * bm))
    total = np.float32(np.sum(np.asarray(partials, dtype=np.float64)) / (B * N))
    return np.asarray(total, dtype=np.float32), res


def kernel(embeddings: np.ndarray, assignments: np.ndarray) -> np.ndarray:
    out, _ = _run(embeddings, assignments, trace=False)
    return out


# revision 49
# speedup vs baseline: 1.0320x; 1.0320x over previous
"""DeepClusteringLoss Trainium2 kernel.

loss = (||V^T V||_F^2 - 2 ||V^T E||_F^2 + ||E^T E||_F^2) / (B*N)
summed over batch, with E = embeddings.reshape(B, N, D), V =
assignments.reshape(B, N, S), N = F*T.

Sharding: data-parallel over batch; one core per batch element; the host
sums the 8 per-core partials (the scalar "all-reduce") and divides by
B*N.

Per-core pipeline (DMA-bound: 23.07 MB fp32 input; the 16 SDMA engines
sustain ~26 GB/s each => ~55-56 us of engine-busy streaming, measured):
- GLOBAL partition map: partition p owns rows [p*1024, (p+1)*1024).
  Chunk c = column c of every partition = 128 rows.
- ALL streaming is HWDGE (SP + ACT rings) in fp32: HWDGE descriptor
  generation is RTL (no Q7 SWDGE boot delay, and ~15% less SDMA
  engine-busy per byte than SWDGE's descriptor format), every DMA
  sprays all 16 SDMA engines evenly, and since the SBUF AXI ports
  (~27 GB/s/engine) bind on the write side, streaming fp32 instead of
  SWDGE-cast-fp16 costs no engine time.
- E streams as column-slices alternating SP/ACT rings through a
  10-deep fp32 ring buffer: small HEAD slices (16,16,32,32) so the
  first ring-FIFO completions (each completion semaphore fires ~3-4 us
  after its data lands) unblock the first casts/matmuls by ~10 us, a
  64-chunk middle, and a tapered tail (48,48,48,32,24,16,8) to shorten
  the post-last-arrival serial chain.  V rides along as three fp32 pieces
  (96/448/480 chunks) cut at slice boundaries: the tiny first piece at
  the very head of the SP ring.
- Interleave copies cast fp32->fp16 while building chunk-PAIR operands
  [V_2q|E_2q|pad4 | V_2q+1|E_2q+1] (128 x 96 fp16, 48-wide halves so
  pair offsets stay 32B-aligned and matmuls are 96 wide, 11% narrower
  than the 108-wide layout): one 4D-AP DVE cast per slice for E, one
  GpSimd cast for V (GpSimd is otherwise idle, issues no DMAs, and
  keeps the ACT sequencer DMA-only so copy waits never block tail E
  DMA issue).  Even/odd Grams accumulate at PSUM partition bases 0/48.
- All DMAs are issued in a first pass with greedy byte-balancing
  across the two rings (so neither ring finishes late), before any
  compute op is emitted — the in-order ACT sequencer can then never
  block DMA issue behind a compute wait.
- The last four slices (80 chunks) retire as 48-wide SINGLES into a
  second PSUM bank, so the main Gram's stop fires four slices early
  and its odd-block epilogue (partition starts must be 0/32/64: pieces
  [32:64) and [64:92)) fully overlaps the tail.  The even block and
  the singles block both live on partitions 0:44, so they share one
  SBUF tile and ONE final 44-descriptor OUT DMA.  The host reassembles
  the four dumped blocks from their disjoint DRAM bands and reduces to
  the scalar partial in float64 (exact).

Remaining fixed costs (measured): ~4.5us front (preamble + TENSOR_LOAD
+ first descriptor gen), ~55.5us stream (engine 0 carries ~4.5us of
instruction fetch and finishes last; every slice spans all 16 engines,
so its lag gates the tail), ~4us completion-semaphore latency on the
last slice, ~2us final cast/matmul/epilogue chain, ~5.5us Tile
end-of-kernel drain + barrier.
"""

import os
from contextlib import ExitStack

import numpy as np

import concourse.bacc as bacc
import concourse.mybir as mybir
import concourse.tile as tile
from concourse.bass_utils import run_bass_kernel_spmd

B, F, T, D, S = 8, 256, 512, 40, 4
N = F * T              # rows per core (131072)
SD = S + D             # 44 combined features
H = 48                 # half-width: V(4) | E(40) | pad(4); 48*2B = 32B-aligned
PW = 2 * H             # paired-chunk width (96)
P = 128                # partitions
U = N // P             # rows per partition in the global map (1024)
N_CORES = 8

MM_DT_NAME = os.environ.get("KERNEL_MM_DT", "float16")
RING = os.environ.get("KERNEL_RING", "alt")   # "alt" | "sp"
EBUFS = int(os.environ.get("KERNEL_EBUFS", "10"))
WBUFS = int(os.environ.get("KERNEL_WBUFS", "8"))

# E slice plan: small HEAD slices so the first ring-FIFO completions
# (and with them the first casts + matmuls) land by ~7 us instead of
# ~16 us; big uniform middle for line-rate DMA; small TAIL taper so the
# last-slice copy+matmul+epilogue dependency chain is short.
SLICES = [16, 16, 32, 32] + [64] * 11 + [48, 48, 48] + [32, 24, 16, 8]
N_SINGLE = 4           # trailing slices retired via the second PSUM bank
assert sum(SLICES) == U
assert all(ub % 2 == 0 for ub in SLICES)

# V pieces, boundaries aligned to slice edges: a tiny leading piece
# covering the head slices (so the first V-copies are not gated by a
# megabyte-scale V transfer), then two big pieces.
VCUTS = [0, 96, 544, U]

_nc_cache = {}


def _build_nc(key):
    (mm_dt_name, ring_mode, ebufs, wbufs) = key
    mm_dt = getattr(mybir.dt, mm_dt_name)
    f32 = mybir.dt.float32

    nc = bacc.Bacc("TRN2", target_bir_lowering=False, debug=False)
    E = nc.dram_tensor("embeddings", (N, D), f32, kind="ExternalInput")
    V = nc.dram_tensor("assignments", (N, S), f32, kind="ExternalInput")
    OUT = nc.dram_tensor("partial", (PW, 176), f32, kind="ExternalOutput")

    # global-map DRAM views: partition p <- rows [p*U, (p+1)*U)
    e_g = E[:, :].rearrange("(p u) d -> p (u d)", p=P)   # [128, U*D]
    v_g = V[:, :].rearrange("(p u) s -> p (u s)", p=P)   # [128, U*S]

    with tile.TileContext(nc) as tc, ExitStack() as ctx:
        res_pool = ctx.enter_context(tc.tile_pool(name="res", bufs=1))
        e_pool = ctx.enter_context(tc.tile_pool(name="e", bufs=ebufs))
        w_pool = ctx.enter_context(tc.tile_pool(name="w", bufs=wbufs))
        psum_pool = ctx.enter_context(tc.tile_pool(name="ps", bufs=1, space="PSUM"))
        g_ps = psum_pool.tile([PW, PW], f32, tag="g")

        # V as three fp32 HWDGE pieces (separate tiles so early slices
        # depend only on the piece that covers them).  The tiny first
        # piece rides at the very head of the SP ring.
        v_tiles = []
        for j in range(len(VCUTS) - 1):
            lo, hi = VCUTS[j], VCUTS[j + 1]
            v_t = res_pool.tile([P, (hi - lo) * S], f32, tag=f"v{j}")
            v_tiles.append((v_t, lo, hi))

        # The final slice accumulates 48-wide SINGLES into a second
        # PSUM bank, so the main Gram's stop fires one slice early and
        # its (bigger) epilogue overlaps the last slice's casts+matmuls.
        g2_ps = psum_pool.tile([H, H], f32, tag="g2")

        # ---- Phase 1: issue ALL DMAs (greedy byte-balanced across the
        # two HWDGE rings, so neither ring finishes late and delays the
        # tail slices).  With every dma_start issued before any ACT
        # copy is emitted, the ACT sequencer's in-order stream can never
        # block DMA issue behind a compute wait, which frees ACT to
        # share the tail cast work with DVE.
        E_CHUNK_B = D * 4          # E bytes per chunk per partition
        V_CHUNK_B = S * 4
        rb = {id(nc.sync): 0, id(nc.scalar): 0}

        def lighter():
            return nc.sync if rb[id(nc.sync)] <= rb[id(nc.scalar)] else nc.scalar

        nc.sync.dma_start(
            out=v_tiles[0][0][:], in_=v_g[:, VCUTS[0] * S:VCUTS[1] * S])
        rb[id(nc.sync)] += (VCUTS[1] - VCUTS[0]) * V_CHUNK_B

        e_tiles = []
        c0 = 0
        for k, ub in enumerate(SLICES):
            e_t = e_pool.tile([P, ub * D], f32, tag="e")
            eng = lighter() if ring_mode == "alt" else nc.sync
            eng.dma_start(out=e_t[:], in_=e_g[:, c0 * D:(c0 + ub) * D])
            rb[id(eng)] += ub * E_CHUNK_B
            if k == 2:
                eng = lighter()
                eng.dma_start(
                    out=v_tiles[1][0][:],
                    in_=v_g[:, VCUTS[1] * S:VCUTS[2] * S])
                rb[id(eng)] += (VCUTS[2] - VCUTS[1]) * V_CHUNK_B
            elif k == 3:
                eng = lighter()
                eng.dma_start(
                    out=v_tiles[2][0][:],
                    in_=v_g[:, VCUTS[2] * S:VCUTS[3] * S])
                rb[id(eng)] += (VCUTS[3] - VCUTS[2]) * V_CHUNK_B
            e_tiles.append((e_t, c0, ub))
            c0 += ub

        # ---- Phase 2: casts + matmuls.  E casts on DVE, V casts on the
        # otherwise-idle GpSimd.  The last N_SINGLE slices retire as
        # 48-wide singles into the second PSUM bank, so the main Gram's
        # stop fires several slices early and its three-piece epilogue
        # fully overlaps the tail slices' casts and matmuls.
        n_sl = len(SLICES)
        pair = 0
        single_u = 0
        n_single_u = sum(SLICES[n_sl - N_SINGLE:])
        for k, (e_t, c0, ub) in enumerate(e_tiles):
            v_src, vlo, vhi = next(
                vt for vt in v_tiles if vt[1] <= c0 < vt[2])
            assert c0 + ub <= vhi
            vc0 = c0 - vlo
            if k < n_sl - N_SINGLE:
                nq = ub // 2
                w_t = w_pool.tile([P, nq * PW], mm_dt, tag="w")
                # 4D views: one cast per slice fills BOTH halves of
                # every pair.
                w5 = w_t[:].rearrange("p (q h c) -> p q h c", h=2, c=H)
                e3 = e_t[:].rearrange("p (q h d) -> p q h d", h=2, d=D)
                v3 = v_src[:, vc0 * S:(vc0 + ub) * S].rearrange(
                    "p (q h s) -> p q h s", h=2, s=S)
                nc.vector.tensor_copy(w5[:, :, :, S:SD], e3)
                nc.gpsimd.tensor_copy(w5[:, :, :, 0:S], v3)
                for q in range(nq):
                    wq = w_t[:, q * PW:(q + 1) * PW]
                    nc.tensor.matmul(
                        g_ps[:], wq, wq,
                        start=(pair == 0),
                        stop=(k == n_sl - N_SINGLE - 1 and q == nq - 1),
                    )
                    pair += 1
            else:
                w_t = w_pool.tile([P, ub * H], mm_dt, tag="w")
                w5 = w_t[:].rearrange("p (u c) -> p u c", c=H)
                e3 = e_t[:].rearrange("p (u d) -> p u d", d=D)
                v3 = v_src[:, vc0 * S:(vc0 + ub) * S].rearrange(
                    "p (u s) -> p u s", s=S)
                nc.vector.tensor_copy(w5[:, :, S:SD], e3)
                nc.gpsimd.tensor_copy(w5[:, :, 0:S], v3)
                for u in range(ub):
                    wu = w_t[:, u * H:(u + 1) * H]
                    nc.tensor.matmul(
                        g2_ps[:], wu, wu,
                        start=(single_u == 0),
                        stop=(single_u == n_single_u - 1))
                    single_u += 1

        # Epilogue: dump only the two 44x44 diagonal Gram blocks of the
        # PSUM accumulator, each on its own HWDGE ring (SP and ACT) so
        # the descriptor generation for the two OUT transfers runs in
        # parallel; the host adds the blocks and reduces to the scalar
        # partial (exact, in float64) alongside the cross-core sum.
        # Partition-start legality: patterns may start at 0/32/64 and,
        # when starting at 32, cover at most 32 partitions.  The odd
        # Gram block lives at [48:92, 48:92], so dump it as two pieces:
        # rows 48:64 ride a 32-partition access at base 32, rows 64:92
        # a 28-partition access at base 64.
        # The even block and the singles block both live on partitions
        # 0:44, so they ride ONE SBUF tile ([44, 88]) and ONE 44-desc
        # OUT DMA -- removing a ~1us descriptor-gen from the critical
        # end.  Each OUT piece gets a disjoint DRAM column band.
        ep = ctx.enter_context(tc.tile_pool(name="ep", bufs=1))
        eg2_sb = ep.tile([SD, 2 * SD], f32, tag="eg2")
        gl_sb = ep.tile([64, SD], f32, tag="gl")
        gh_sb = ep.tile([92, SD], f32, tag="gh")
        # All epilogue copies on DVE (idle in this window): using
        # nc.scalar.copy here would pull in ACT_TABLE_LOAD, whose ~19KB
        # table fetch rides queue 14 on straggler engine 0.
        nc.vector.tensor_copy(eg2_sb[:, 0:SD], g_ps[0:SD, 0:SD])
        nc.vector.tensor_copy(gl_sb[32:64, :], g_ps[32:64, H:H + SD])
        nc.vector.tensor_copy(gh_sb[64:92, :], g_ps[64:92, H:H + SD])
        nc.scalar.dma_start(out=OUT[32:64, 88:132], in_=gl_sb[32:64, :])
        nc.scalar.dma_start(out=OUT[64:92, 132:176], in_=gh_sb[64:92, :])
        nc.vector.tensor_copy(eg2_sb[:, SD:2 * SD], g2_ps[0:SD, 0:SD])
        nc.sync.dma_start(out=OUT[0:SD, 0:2 * SD], in_=eg2_sb[:])

    nc.finalize()
    return nc


def _get_nc():
    key = (MM_DT_NAME, RING, EBUFS, WBUFS)
    if key not in _nc_cache:
        _nc_cache[key] = _build_nc(key)
    return _nc_cache[key]


def _run(embeddings: np.ndarray, assignments: np.ndarray, trace: bool = False):
    nc = _get_nc()
    in_maps = []
    for i in range(N_CORES):
        in_maps.append({
            "embeddings": np.ascontiguousarray(
                embeddings[i].reshape(N, D).astype(np.float32, copy=False)),
            "assignments": np.ascontiguousarray(
                assignments[i].reshape(N, S).astype(np.float32, copy=False)),
        })
    try:
        res = run_bass_kernel_spmd(
            nc, in_maps, core_ids=list(range(N_CORES)), trace=trace
        )
    except Exception:
        res = run_bass_kernel_spmd(
            nc, in_maps, core_ids=list(range(N_CORES)), trace=trace
        )
    partials = []
    for r in res.results:
        gp = np.asarray(r["partial"], dtype=np.float64)
        # even block + singles block (merged dump), + the odd block's
        # two pieces from their relocated DRAM bands.
        odd = np.empty((SD, SD))
        odd[0:16] = gp[48:64, 88:132]
        odd[16:SD] = gp[64:64 + 28, 132:176]
        G = gp[0:SD, 0:SD] + gp[0:SD, SD:2 * SD] + odd
        bm = G[0:S, S:SD]
        partials.append(np.sum(G * G) - 4.0 * np.sum(bm * bm))
    total = np.float32(np.sum(np.asarray(partials, dtype=np.float64)) / (B * N))
    return np.asarray(total, dtype=np.float32), res


def kernel(embeddings: np.ndarray, assignments: np.ndarray) -> np.ndarray:
    out, _ = _run(embeddings, assignments, trace=False)
    if not np.isfinite(out):
        # transient HW fault (seen once in ~100 runs under heavy
        # co-tenant load): re-dispatch once
        out, _ = _run(embeddings, assignments, trace=False)
    return out
